# revision 2
# baseline (speedup 1.0000x reference)
"""Distributed causal-attention kernel for one TRN2 chip (8 NeuronCores).

Sharding (hardcoded): core i handles batch i//4 and head-group i%4
(2 heads of 8, head_dim 128).  Each core:
  RMSNorm(x_b) -> QKV proj (its heads) -> causal attention -> O^T
  -> partial output projection y^T_partial = sum_h Wout_h^T O_h^T
     for the FULL sequence of its batch (contribution of its 2 heads).
Host folds gamma + attention scale into the pre-transposed weights and
sums the 4 per-head-group partials of each batch during unsharding
(the reduction IS the gather for row-split to_out).  No collectives.
"""

import numpy as np

import concourse.bass as bass
import concourse.mybir as mybir
import concourse.tile as tile
from concourse import bacc
from concourse.bass_utils import run_bass_kernel_spmd
from concourse.masks import make_identity

F32 = mybir.dt.float32
F16 = mybir.dt.float16
BF = mybir.dt.bfloat16
AF = mybir.ActivationFunctionType

S = 2048          # sequence length
D = 1024          # model dim
DH = 128          # head dim
HC = 2            # heads per core
FQKV = 3 * HC * DH  # 768 qkv cols per core (pre-transposed layout)
P = 128
SB = S // P       # 16 seq blocks
KD = D // P       # 8 d blocks
SA = float(DH) ** -0.5
NEG = -30000.0    # causal mask bias (exp underflows to exactly 0)


def _body(tc):
    nc = tc.nc
    x_ext = nc.declare_dram_parameter("x", [S, D], BF, isOutput=False)
    wqkv_ext = nc.declare_dram_parameter("w_qkvT", [D, FQKV], BF, isOutput=False)
    wout_ext = nc.declare_dram_parameter("w_outT", [DH, HC, D], BF, isOutput=False)
    out_ext = nc.declare_dram_parameter("out", [D, S], F16, isOutput=True)

    from contextlib import ExitStack
    with ExitStack() as ctx:
        wpool = ctx.enter_context(tc.tile_pool(name="wpool", bufs=1))
        wqkvT = wpool.tile([P, KD, FQKV], BF)
        nc.scalar.dma_start(
            wqkvT, wqkv_ext.ap().rearrange("(o p) f -> p o f", p=P))
        woT = wpool.tile([P, HC, D], BF)
        nc.scalar.dma_start(woT, wout_ext.ap())

        const = ctx.enter_context(tc.tile_pool(name="const", bufs=1))
        dram = ctx.enter_context(tc.tile_pool(name="dram", bufs=1, space="DRAM"))
        big = ctx.enter_context(tc.tile_pool(name="big", bufs=1))
        xload = ctx.enter_context(tc.tile_pool(name="xload", bufs=8))
        cast = ctx.enter_context(tc.tile_pool(name="cast", bufs=4))
        stat = ctx.enter_context(tc.tile_pool(name="stat", bufs=8))
        ptp = ctx.enter_context(tc.tile_pool(name="ptp", bufs=6))
        yout = ctx.enter_context(tc.tile_pool(name="yout", bufs=4))
        ps_mm = ctx.enter_context(tc.tile_pool(name="ps_mm", bufs=2, space="PSUM"))
        ps_s = ctx.enter_context(tc.tile_pool(name="ps_s", bufs=3, space="PSUM"))
        ps_o = ctx.enter_context(tc.tile_pool(name="ps_o", bufs=2, space="PSUM"))
        ps_l = ctx.enter_context(tc.tile_pool(name="ps_l", bufs=1, space="PSUM"))

        # ---- constants ----
        ident = const.tile([P, P], BF)
        make_identity(nc, ident)
        masksT = []
        for t in range(4):
            mk = const.tile([P, 512], BF, tag=f"maskT{t}")
            if t > 0:
                nc.gpsimd.memset(mk[:, : t * P], NEG)
            # keep (0) where i >= j inside the diagonal block
            nc.gpsimd.memset(mk[:, t * P:(t + 1) * P], 0.0)
            nc.gpsimd.affine_select(
                out=mk[:, t * P:(t + 1) * P],
                in_=mk[:, t * P:(t + 1) * P],
                compare_op=mybir.AluOpType.is_ge,
                fill=NEG, base=0,
                pattern=[[1, P]], channel_multiplier=-1)
            if t < 3:
                nc.gpsimd.memset(mk[:, (t + 1) * P:], 0.0)
            masksT.append(mk)

        ones_bf = const.tile([P, 1], BF)
        nc.vector.memset(ones_bf, 1.0)
        ones_row = const.tile([1, P], BF)
        nc.vector.memset(ones_row, 1.0)

        # ---- persistent activations ----
        xn_dram = dram.tile([S, D], BF)
        xnT = big.tile([P, KD, S], BF)
        qkvT = big.tile([P, 6, S], BF)
        v_sb = big.tile([P, SB, HC * DH], BF)
        oS = big.tile([P, HC, 4, 512], BF)   # O^T per (head, seq superblock)

        # ---- per 512-chunk: norm -> transpose -> QKV -> V ----
        for c in range(4):
            # norm: xn = x * (32/||x||) for s-blocks of this chunk
            xts = []
            ssg = stat.tile([P, 4], F32, tag="ssg")
            for j in range(4):
                si = c * 4 + j
                xt = xload.tile([P, D], BF, tag="xt")
                nc.sync.dma_start(xt, x_ext[si * P:(si + 1) * P, :])
                sq = cast.tile([P, D], BF, tag="sq")
                nc.scalar.activation(sq, xt, AF.Square,
                                     accum_out=ssg[:, j:j + 1])
                xts.append(xt)
            slg = stat.tile([P, 4], F32, tag="slg")
            nc.scalar.activation(slg, ssg, AF.Sqrt, scale=1.0 / D)
            scg = stat.tile([P, 4], F32, tag="scg")
            nc.vector.reciprocal(scg, slg)
            for j in range(4):
                si = c * 4 + j
                xnb = cast.tile([P, D], BF, tag="xnb")
                nc.vector.tensor_scalar_mul(xnb, xts[j], scg[:, j:j + 1])
                nc.scalar.dma_start(xn_dram[si * P:(si + 1) * P, :], xnb)
            # transpose chunk back: xnT [d_inner, d_outer, s]
            for k in range(KD):
                nc.sync.dma_start_transpose(
                    xnT[:, k, c * 512:(c + 1) * 512],
                    xn_dram[c * 512:(c + 1) * 512, k * P:(k + 1) * P])
            # QKV projection for this chunk (pairs share ldweights)
            for fp in range(3):
                pms = [ps_mm.tile([P, 512], F32, tag="pm", name=f"pm{u}")
                       for u in range(2)]
                for k in range(KD):
                    for u in range(2):
                        fb = fp * 2 + u
                        nc.tensor.matmul(
                            pms[u], wqkvT[:, k, fb * P:(fb + 1) * P],
                            xnT[:, k, c * 512:(c + 1) * 512],
                            start=(k == 0), stop=(k == KD - 1))
                for u in range(2):
                    fb = fp * 2 + u
                    nc.vector.tensor_copy(
                        qkvT[:, fb, c * 512:(c + 1) * 512], pms[u])
            # V natural layout for this chunk
            for h in range(HC):
                pst = ps_mm.tile([P, 512], BF, tag="pm")
                for j in range(4):
                    sb = c * 4 + j
                    nc.tensor.transpose(
                        pst[:, j * P:(j + 1) * P],
                        qkvT[:, 4 + h, sb * P:(sb + 1) * P], ident)
                nc.vector.tensor_copy(
                    v_sb[:, c * 4:(c + 1) * 4, h * DH:(h + 1) * DH],
                    pst.rearrange("p (j q) -> p j q", j=4))

        # ---- attention: S^T = K^T-block x Q^T -> exp -> PV, 1/l at the end
        def attn_super(h, a):
            po = ps_o.tile([P, 512], F32, tag="po", name=f"po{h}_{a}")
            lp = ps_l.tile([1, 512], F32, tag="lp", name=f"lp{h}_{a}")
            nj = 4 * (a + 1)
            for jb in range(nj):
                t = jb - 4 * a
                ps = ps_s.tile([P, 512], F32, tag="s", name=f"ps{h}_{a}_{jb}")
                nc.tensor.matmul(
                    ps, qkvT[:, 2 + h, jb * P:(jb + 1) * P],
                    qkvT[:, h, a * 512:(a + 1) * 512],
                    start=True, stop=(t < 0))
                if t >= 0:
                    nc.tensor.matmul(ps, ident, masksT[t],
                                     start=False, stop=True)
                ptt = ptp.tile([P, 512], BF, tag="ptt", name=f"ptt{h}_{a}_{jb}")
                nc.scalar.activation(ptt, ps, AF.Exp)
                nc.tensor.matmul(lp, ones_bf, ptt,
                                 start=(jb == 0), stop=(jb == nj - 1))
                nc.tensor.matmul(
                    po, v_sb[:, jb, h * DH:(h + 1) * DH], ptt,
                    start=(jb == 0), stop=(jb == nj - 1))
            rl = stat.tile([1, 512], BF, tag="rl")
            with nc.allow_low_precision(reason="bf16 1/l bcast"):
                nc.vector.reciprocal(rl, lp)
            rlps = ps_mm.tile([P, 512], F32, tag="pm", name=f"rlps{h}_{a}")
            nc.tensor.matmul(rlps, ones_row, rl, start=True, stop=True)
            rlb = cast.tile([P, 512], F32, tag="rlb")
            nc.vector.tensor_copy(rlb, rlps)
            nc.vector.tensor_mul(oS[:, h, a], po, rlb)

        # a-outer / h-inner so the partial out-projection for superblock a
        # can stream to DRAM while later superblocks still compute
        for a in range(4):
            for h in range(HC):
                attn_super(h, a)
            # partial out-proj for this 512-seq chunk: y^T[c,:] += sum_h
            for cp in range(4):
                pms = [ps_mm.tile([P, 512], F32, tag="pm", name=f"pmo{u}")
                       for u in range(2)]
                for u in range(2):
                    cb = cp * 2 + u
                    for h in range(HC):
                        nc.tensor.matmul(
                            pms[u], woT[:, h, cb * P:(cb + 1) * P],
                            oS[:, h, a],
                            start=(h == 0), stop=(h == HC - 1))
                for u in range(2):
                    cb = cp * 2 + u
                    y = yout.tile([P, 512], F16, tag="y")
                    nc.vector.tensor_copy(y, pms[u])
                    nc.sync.dma_start(
                        out_ext[cb * P:(cb + 1) * P,
                                a * 512:(a + 1) * 512], y)


def build():
    nc = bacc.Bacc(None, target_bir_lowering=False)
    with tile.TileContext(nc) as tc:
        _body(tc)
    nc.compile()
    return nc


_NC = None


def make_in_maps(inputs):
    import ml_dtypes
    x = np.ascontiguousarray(np.asarray(inputs["x"], np.float32))
    gamma = np.asarray(inputs["gamma"], np.float32)
    w_qkv = np.asarray(inputs["w_qkv"], np.float32)
    w_out = np.asarray(inputs["w_out"], np.float32)
    w_prep = w_qkv * gamma[None, :]          # fold RMSNorm gamma
    in_maps = []
    for i in range(8):
        b, g = i // 4, i % 4
        rows = np.concatenate([
            w_prep[256 * g:256 * (g + 1)] * SA,   # fold attn scale into Q
            w_prep[1024 + 256 * g:1024 + 256 * (g + 1)],
            w_prep[2048 + 256 * g:2048 + 256 * (g + 1)]], axis=0)
        # w_out columns for this head group, laid out [dh, h, c_out]
        wo = w_out[:, 256 * g:256 * (g + 1)].T.reshape(HC, DH, D)
        wo = np.ascontiguousarray(wo.transpose(1, 0, 2))
        in_maps.append({
            "x": np.ascontiguousarray(x[b]).astype(ml_dtypes.bfloat16),
            "w_qkvT": np.ascontiguousarray(rows.T).astype(ml_dtypes.bfloat16),
            "w_outT": wo.astype(ml_dtypes.bfloat16)})
    return in_maps


def run(inputs, trace=False):
    global _NC
    if _NC is None:
        _NC = build()
    in_maps = make_in_maps(inputs)
    br = run_bass_kernel_spmd(_NC, in_maps, list(range(8)), trace=trace)
    out = np.empty((2, S, D), np.float32)
    for b in range(2):
        acc = np.zeros((D, S), np.float32)
        for g in range(4):
            acc += np.asarray(br.results[4 * b + g]["out"], np.float32)
        out[b] = acc.T
    return out, br


def kernel(**inputs):
    out, _ = run(inputs, trace=False)
    return out
